# revision 26
# baseline (speedup 1.0000x reference)
"""2-layer GAT (PyG GATConv semantics) on 8 Trainium2 NeuronCores.

v2 "streamed subtiles" design. Key facts driving it (HW-measured):
- dma_gather costs ~8.4 ns per index (descriptor) regardless of element
  size (256B..2KB), locality, or queue count -> minimize gathered slots.
- Tables stored bf16: row1 = 128 bf16 (256B elem) = [h(64)|asrc(2)|adst(2)],
  row2 = 256 bf16 (512B elem) = [h2(128)|asrc2|adst2].
- Edges packed into single-destination rows of nc_b columns, streamed
  across per-(group,bank) subtile sequences; quads bleed across subtile
  boundaries (row offsets R̄ maxed over cores so the instruction schedule
  is core-invariant SPMD).
- One-hot combine (P1) / adst-broadcast (P2) matrices are host-built bf16
  constants DMA'd in (DMA engines are idle; DVE is not).
- Per-(group,bank): one gather, one P12 load, a few matmuls, batched
  ACT/DVE ops; per-quad: ~6 combine matmuls + finalize.
"""

import os

import numpy as np
import ml_dtypes

import concourse.bacc as bacc
import concourse.mybir as mybir
import concourse.tile as tile
from concourse.bass_utils import run_bass_kernel_spmd

F32 = mybir.dt.float32
BF16 = mybir.dt.bfloat16
I16 = mybir.dt.int16
AF = mybir.ActivationFunctionType
ALU = mybir.AluOpType
BF = ml_dtypes.bfloat16

NCORES = 8
BANK = 32768
NEG = 0.2
QR = 128
NPQ = 112            # nodes per quad (rows stay 128)
FIN = 128
H1, FH = 2, 32
D1 = H1 * FH         # 64
FOUT = 128
ROW1 = 128           # f32: [h(64)|asrc(2)|adst(2)|pad] -> 512B elem
ROW2 = 256           # bf16: [h2(128)|asrc2|adst2|pad]   -> 512B
GRP = 8              # quads per group


def _wrap_idx(idx):
    n = idx.shape[0]
    assert n % 16 == 0
    blk = idx.reshape(n // 16, 16).T.astype(np.int16)
    return np.tile(blk, (8, 1))


def _groups(qper):
    return [list(range(g, min(g + GRP, qper))) for g in range(0, qper, GRP)]


def preprocess(x, edge_index, W1, att_src1, att_dst1, b1, W2, att_src2,
               att_dst2, b2):
    N = x.shape[0]
    # The implicit self-loop PyG adds is handled locally on-chip, not
    # gathered. Explicit self-edges in the input stay in the edge list.
    src = np.asarray(edge_index[0], dtype=np.int64)
    dst = np.asarray(edge_index[1], dtype=np.int64)

    nquads = (N + NPQ - 1) // NPQ
    nquads = ((nquads + NCORES - 1) // NCORES) * NCORES
    NP = nquads * QR
    qper = nquads // NCORES
    shard = NP // NCORES
    # chunks of quads (k-ranges); chunk ch = table bank ch. Rows per bank =
    # NCORES * q_ch * 128 must stay within int16 gather reach (<= 32768).
    nbanks = max(1, -(-qper // 31))
    q_ch = [len(a) for a in np.array_split(np.arange(qper), nbanks)]
    k_start = np.concatenate([[0], np.cumsum(q_ch)]).astype(np.int64)
    bank_rows = [NCORES * qc * QR for qc in q_ch]
    bank_start = np.concatenate([[0], np.cumsum(bank_rows)]).astype(np.int64)
    pad_rows = [int(bank_start[b + 1] - 1) for b in range(nbanks)]

    deg = np.bincount(dst, minlength=N).astype(np.int64)

    # --- greedy LPT node->quad packing (equal edges per quad) ---
    import heapq
    qcount = np.zeros(nquads, dtype=np.int64)
    qload = np.zeros(nquads, dtype=np.int64)
    heap = [(0, 0, q) for q in range(nquads)]
    heapq.heapify(heap)
    node_quad = np.empty(N, dtype=np.int64)
    for n in np.argsort(-deg, kind="stable"):
        while True:
            _, _, q = heapq.heappop(heap)
            if qcount[q] < NPQ:
                break
        node_quad[n] = q
        qcount[q] += 1
        qload[q] += deg[n]
        heapq.heappush(heap, (qload[q], qcount[q], q))

    # out_of_node: q*128 + r (per-core output layout, r in [0, NPQ))
    # tbl_of_node: chunk-interleaved AllGather table layout
    out_of_node = np.empty(N, dtype=np.int64)
    nodes_by_quad = [[] for _ in range(nquads)]
    for n in range(N):
        nodes_by_quad[node_quad[n]].append(n)
    for q in range(nquads):
        for r, n in enumerate(nodes_by_quad[q]):
            out_of_node[n] = q * QR + r
    oq = out_of_node // QR
    oc = oq // qper
    ok = oq % qper
    orr = out_of_node % QR
    och = np.searchsorted(k_start, ok, side="right") - 1
    tbl_of_node = (bank_start[och] + oc * (QR * np.asarray(q_ch)[och])
                   + (ok - k_start[och]) * QR + orr)

    psrc = tbl_of_node[src]
    pdst = out_of_node[dst]
    ebank = np.searchsorted(bank_start, psrc, side="right") - 1

    # --- per (packed dst row, bank) segments ---
    key = pdst * nbanks + ebank
    order = np.argsort(key, kind="stable")
    skey = key[order]
    ssrc = psrc[order]
    ukey, ustart, ucnt = np.unique(skey, return_index=True,
                                   return_counts=True)
    # ukey = pdst*nbanks + b; derive quad and core of each segment
    seg_pdst = ukey // nbanks
    seg_bank = ukey % nbanks
    seg_quad = seg_pdst // QR

    # --- choose nc_b by exact cost over candidate widths ---
    # rows[c,k,b](nc) = sum over segs of ceil(cnt/nc); cost = slots*8.4ns +
    # subtile fixed cost.
    ncs = {}
    rows_cache = {}
    for b in range(nbanks):
        m = seg_bank == b
        sq = seg_quad[m]
        sc = ucnt[m]
        best = None
        for nc in range(2, 8):
            r = (sc + nc - 1) // nc
            rows_qb = np.bincount(sq, weights=r.astype(np.float64),
                                  minlength=nquads).astype(np.int64)
            # R̄ per slot k = max over cores
            rkb = rows_qb.reshape(NCORES, qper).max(axis=0)
            tot_rows = int(rkb.sum())
            slots = tot_rows * nc * QR // QR  # rows * nc slots each
            cost = tot_rows * nc * 8.4 + (tot_rows / 128.0) * 1100.0
            if best is None or cost < best[0]:
                best = (cost, nc, rkb)
        _, nc, rkb = best
        ncs[b] = nc
        rows_cache[b] = rkb

    # recompute exact per-(core,quad) rows for chosen ncs
    rows_ckb = np.zeros((nquads, nbanks), dtype=np.int64)
    for b in range(nbanks):
        m = seg_bank == b
        r = (ucnt[m] + ncs[b] - 1) // ncs[b]
        rows_ckb[:, b] = np.bincount(seg_quad[m],
                                     weights=r.astype(np.float64),
                                     minlength=nquads).astype(np.int64)
    Rbar = rows_ckb.reshape(NCORES, qper, nbanks).max(axis=0)  # [qper,nbanks]

    groups = _groups(qper)
    # per (g, b): quad row offsets, S̄, incidence schedule (core-invariant)
    sched = []   # per (g,b): dict(offs, S, incs=[(s, kk, first_of_s, last)])
    for gi, grp in enumerate(groups):
        for b in range(nbanks):
            offs = np.zeros(len(grp) + 1, dtype=np.int64)
            for j, k in enumerate(grp):
                offs[j + 1] = offs[j] + Rbar[k, b]
            S = int((offs[-1] + QR - 1) // QR)
            incs = []
            for s in range(S):
                lo, hi = s * QR, (s + 1) * QR
                kks = [j for j in range(len(grp))
                       if offs[j] < hi and offs[j + 1] > lo]
                for t, j in enumerate(kks):
                    incs.append((s, j, t == 0, t == len(kks) - 1))
            sched.append({"offs": offs, "S": S, "incs": incs})

    nc_list = [ncs[b] for b in range(nbanks)]
    # idx/p1/p2 blob column layouts (shared across cores)
    idx_off = [0]
    p1_off = [0]
    for gi, grp in enumerate(groups):
        for b in range(nbanks):
            sc = sched[gi * nbanks + b]
            idx_off.append(idx_off[-1] + sc["S"] * nc_list[b] * QR)
            p1_off.append(p1_off[-1] + len(sc["incs"]) * QR)
    n_idx = idx_off[-1]
    n_p1 = p1_off[-1]

    # --- per-core blobs ---
    W1 = np.asarray(W1, dtype=np.float32)
    W2 = np.asarray(W2, dtype=np.float32)
    a_s1 = np.asarray(att_src1, dtype=np.float32)
    a_d1 = np.asarray(att_dst1, dtype=np.float32)
    a_s2 = np.asarray(att_src2, dtype=np.float32)
    a_d2 = np.asarray(att_dst2, dtype=np.float32)
    W1a_s = np.stack([W1[:, h * FH:(h + 1) * FH] @ a_s1[h]
                      for h in range(H1)], 1)
    W1a_d = np.stack([W1[:, h * FH:(h + 1) * FH] @ a_d1[h]
                      for h in range(H1)], 1)
    Wext1 = np.concatenate([W1, W1a_s, W1a_d], axis=1)          # [FIN, 68]
    Wext2 = np.concatenate([W2, (W2 @ a_s2[0])[:, None],
                            (W2 @ a_d2[0])[:, None]], axis=1)   # [D1, 130]
    b1e = np.zeros((1, D1 + H1), dtype=np.float32)
    b1e[0, :D1] = b1
    b2e = np.zeros((1, FOUT + 1), dtype=np.float32)
    b2e[0, :FOUT] = b2
    padrow1 = np.zeros((1, ROW1), dtype=np.float32)
    padrow1[0, D1:D1 + H1] = -300.0  # stays f32
    padrow2 = np.zeros((1, ROW2), dtype=np.float32)
    padrow2[0, FOUT] = -300.0

    xT = np.zeros((FIN, NP), dtype=BF)
    xT[:, out_of_node] = np.asarray(x, dtype=np.float32).T.astype(BF)

    const = {
        "Wext1": Wext1.astype(BF), "Wext2": Wext2.astype(BF),
        "b1e": b1e.astype(BF), "b2e": b2e.astype(BF),
        "ones_row": np.ones((1, QR), dtype=BF),
        "ident": np.eye(QR, dtype=np.float32).astype(BF),
        "padrow1": padrow1, "padrow2": padrow2.astype(BF),
    }

    # per-segment source lists, ordered by (pdst, bank)
    in_maps = []
    for c in range(NCORES):
        idx_blob = np.empty(n_idx, dtype=np.int16)
        p1_blob = np.zeros((QR, n_p1), dtype=BF)
        p2_blob = np.zeros((QR, n_p1), dtype=BF)
        blk = 0
        for gi, grp in enumerate(groups):
            for b in range(nbanks):
                sc = sched[gi * nbanks + b]
                nc = nc_list[b]
                S, offs = sc["S"], sc["offs"]
                padi = pad_rows[b] - int(bank_start[b])
                ncols = S * nc
                idx2 = np.full((ncols, QR), padi, dtype=np.int16)
                dstg = np.full(S * QR, -1, dtype=np.int64)
                for j, k in enumerate(grp):
                    q = c * qper + k
                    # segments of quad q bank b
                    lo = np.searchsorted(ukey, q * QR * nbanks)
                    hi = np.searchsorted(ukey, (q + 1) * QR * nbanks)
                    r = int(offs[j])
                    for si in range(lo, hi):
                        if seg_bank[si] != b:
                            continue
                        d = seg_pdst[si] % QR
                        st0 = ustart[si]
                        cnt = ucnt[si]
                        srcs = (ssrc[st0:st0 + cnt]
                                - int(bank_start[b]))
                        pos = 0
                        while pos < cnt:
                            take = min(nc, cnt - pos)
                            s_i, r_i = r // QR, r % QR
                            idx2[s_i * nc:s_i * nc + take, r_i] = \
                                srcs[pos:pos + take]
                            dstg[r] = j * QR + d
                            pos += take
                            r += 1
                    assert r <= offs[j + 1], (c, gi, b, j, r, offs)
                    # rows offs[j]..offs[j+1) beyond r stay pad (-1)
                idx_blob[idx_off[blk]:idx_off[blk + 1]] = idx2.reshape(-1)
                # incidences -> P1/P2
                for i, (s, j, _, _) in enumerate(sc["incs"]):
                    dsub = dstg[s * QR:(s + 1) * QR]
                    rr = np.nonzero((dsub >= j * QR) &
                                    (dsub < (j + 1) * QR))[0]
                    mm = (dsub[rr] - j * QR).astype(np.int64)
                    base = p1_off[blk] + i * QR
                    P1 = np.zeros((QR, QR), dtype=BF)
                    P1[rr, mm] = 1
                    p1_blob[:, base:base + QR] = P1
                    p2_blob[:, base:base + QR] = P1.T
                blk += 1
        im = dict(const)
        im["xT"] = np.ascontiguousarray(xT[:, c * shard:(c + 1) * shard])
        im["gidx"] = _wrap_idx(idx_blob)
        im["p1"] = p1_blob
        im["p2"] = p2_blob
        in_maps.append(im)

    meta = {
        "N": N, "NP": NP, "qper": qper, "shard": shard, "nbanks": nbanks,
        "pad_rows": pad_rows, "bank_rows": bank_rows,
        "bank_start": [int(v) for v in bank_start], "q_ch": q_ch,
        "k_start": [int(v) for v in k_start],
        "packed_of_node": out_of_node, "nc_list": nc_list,
        "sched": sched, "idx_off": idx_off, "p1_off": p1_off,
        "n_idx": n_idx, "n_p1": n_p1,
    }
    return in_maps, meta


def build(nc, meta):
    qper, nbanks = meta["qper"], meta["nbanks"]
    NP, shard = meta["NP"], meta["shard"]
    pad_rows, bank_rows = meta["pad_rows"], meta["bank_rows"]
    nc_list, sched = meta["nc_list"], meta["sched"]
    idx_off = meta["idx_off"]
    groups = _groups(qper)

    xT_in = nc.dram_tensor("xT", [FIN, shard], BF16, kind="ExternalInput")
    Wext1_in = nc.dram_tensor("Wext1", [FIN, D1 + 2 * H1], BF16,
                              kind="ExternalInput")
    Wext2_in = nc.dram_tensor("Wext2", [D1, FOUT + 2], BF16,
                              kind="ExternalInput")
    b1e_in = nc.dram_tensor("b1e", [1, D1 + H1], BF16, kind="ExternalInput")
    b2e_in = nc.dram_tensor("b2e", [1, FOUT + 1], BF16, kind="ExternalInput")
    ones_in = nc.dram_tensor("ones_row", [1, QR], BF16, kind="ExternalInput")
    ident_in = nc.dram_tensor("ident", [QR, QR], BF16, kind="ExternalInput")
    pr1_in = nc.dram_tensor("padrow1", [1, ROW1], F32, kind="ExternalInput")
    pr2_in = nc.dram_tensor("padrow2", [1, ROW2], BF16, kind="ExternalInput")
    gidx_in = nc.dram_tensor("gidx", [QR, meta["n_idx"] // 16], I16,
                             kind="ExternalInput")
    p1_in = nc.dram_tensor("p1", [QR, meta["n_p1"]], BF16,
                           kind="ExternalInput")
    p2_in = nc.dram_tensor("p2", [QR, meta["n_p1"]], BF16,
                           kind="ExternalInput")
    out_ext = nc.dram_tensor("out", [shard, FOUT], F32, kind="ExternalOutput")

    with tile.TileContext(nc) as tc:
        with tc.tile_pool(name="dram", bufs=1, space="DRAM") as dr:
            bounceA = dr.tile([shard, ROW1], F32)
            table1 = dr.tile([NP, ROW1], F32)
            bounceB = dr.tile([shard, ROW2], BF16)
            table2 = dr.tile([NP, ROW2], BF16)

            with tc.tile_pool(name="const", bufs=1) as cst:
                Wext1_t = cst.tile([FIN, D1 + 2 * H1], BF16)
                Wext2_t = cst.tile([D1, FOUT + 2], BF16)
                b1e_t = cst.tile([1, D1 + H1], BF16)
                b2e_t = cst.tile([1, FOUT + 1], BF16)
                ones_t = cst.tile([1, QR], BF16)
                ident_t = cst.tile([QR, QR], BF16)
                pr1_t = cst.tile([1, ROW1], F32)
                pr2_t = cst.tile([1, ROW2], BF16)
                for t, s in [(Wext1_t, Wext1_in), (Wext2_t, Wext2_in),
                             (b1e_t, b1e_in), (b2e_t, b2e_in),
                             (ones_t, ones_in), (ident_t, ident_in),
                             (pr1_t, pr1_in), (pr2_t, pr2_in)]:
                    nc.sync.dma_start(t[:], s[:])

                bA_r = bounceA[:].rearrange("(q p) c -> p q c", p=QR)
                bB_r = bounceB[:].rearrange("(q p) c -> p q c", p=QR)
                out_r = out_ext[:].rearrange("(q p) c -> p q c", p=QR)

                # ---------- phase A: bounceA = [x@W1 | asrc1 | adst1] -------
                # chunked AllGathers issued as soon as each chunk's quads
                # are written, so AG overlaps phase A and layer 1.
                k_start = meta["k_start"]
                bank_start = meta["bank_start"]
                issue_after = {}
                for b in range(nbanks):
                    gg = (k_start[b + 1] + GRP - 1) // GRP - 1
                    issue_after.setdefault(gg, []).append(b)

                def _ag(tbl, bounce, prt, row, b):
                    nc.gpsimd.collective_compute(
                        "AllGather", ALU.bypass,
                        replica_groups=[list(range(NCORES))],
                        ins=[bounce[k_start[b] * QR:
                                    k_start[b + 1] * QR, :].opt()],
                        outs=[tbl[bank_start[b]:bank_start[b + 1], :].opt()])
                    nc.scalar.dma_start(
                        tbl[pad_rows[b]:pad_rows[b] + 1, :], prt[:])

                with (
                    tc.tile_pool(name="pa", bufs=3) as pa,
                    tc.tile_pool(name="pa_ps", bufs=2, space="PSUM") as pa_ps,
                ):
                    for gi, grp in enumerate(groups):
                        ng = len(grp)
                        xTt = pa.tile([FIN, ng * QR], BF16, tag="xT")
                        nc.sync.dma_start(
                            xTt[:],
                            xT_in[:, grp[0] * QR:(grp[-1] + 1) * QR])
                        stg = pa.tile([QR, ng, D1 + 2 * H1], F32, tag="stA")
                        for j in range(ng):
                            ps = pa_ps.tile([QR, D1 + 2 * H1], F32, tag="psA")
                            nc.tensor.matmul(
                                ps[:], xTt[:, j * QR:(j + 1) * QR],
                                Wext1_t[:], start=True, stop=True)
                            nc.scalar.copy(stg[:, j, :], ps[:])
                        nc.scalar.dma_start(
                            bA_r[:, grp[0]:grp[-1] + 1, 0:D1 + 2 * H1],
                            stg[:])
                        for b in issue_after.get(gi, []):
                            _ag(table1, bounceA, pr1_t, ROW1, b)

                _emit_layer(
                    nc, tc, meta, groups, layer=1, table=table1,
                    gdt=F32, row_w=ROW1, feat=D1, heads=H1,
                    adst_off=D1 + H1,
                    adst_src_r=bA_r, gidx_in=gidx_in, p1_in=p1_in, p2_in=p2_in,
                    ones_t=ones_t, bias_t=b1e_t, ident_t=ident_t,
                    Wext2_t=Wext2_t, bB_r=bB_r, out_r=None,
                    ag_hook=lambda gi: [_ag(table2, bounceB, pr2_t, ROW2, b)
                                        for b in issue_after.get(gi, [])])

                # table2 AG chunks are issued inside layer 1's group loop

                _emit_layer(
                    nc, tc, meta, groups, layer=2, table=table2,
                    gdt=BF16, row_w=ROW2, feat=FOUT, heads=1,
                    adst_off=FOUT + 1,
                    adst_src_r=bB_r, gidx_in=gidx_in, p1_in=p1_in, p2_in=p2_in,
                    ones_t=ones_t, bias_t=b2e_t, ident_t=ident_t,
                    Wext2_t=None, bB_r=None, out_r=out_r)
    return nc


def _emit_layer(nc, tc, meta, groups, layer, table, gdt, row_w, feat, heads,
                adst_off, adst_src_r, gidx_in, p1_in, p2_in, ones_t, bias_t,
                ident_t, Wext2_t, bB_r, out_r, ag_hook=None):
    qper, nbanks = meta["qper"], meta["nbanks"]
    bank_rows, nc_list = meta["bank_rows"], meta["nc_list"]
    sched, idx_off, p1_off = meta["sched"], meta["idx_off"], meta["p1_off"]
    ocols = feat + heads
    hw = feat // heads
    maxS = max(sc["S"] for sc in sched)

    with (
        tc.tile_pool(name=f"gL{layer}", bufs=3) as gp,
        tc.tile_pool(name=f"ixL{layer}", bufs=4) as ixp,
        tc.tile_pool(name=f"pL{layer}", bufs=nbanks + 1) as pp,
        tc.tile_pool(name=f"p2L{layer}", bufs=4) as pp2,
        tc.tile_pool(name=f"oL{layer}", bufs=2) as op,
        tc.tile_pool(name=f"wL{layer}", bufs=3) as wp,
        tc.tile_pool(name=f"adL{layer}", bufs=2, space="PSUM") as ad_ps,
        tc.tile_pool(name=f"cmbL{layer}", bufs=2, space="PSUM") as cmb_ps,
        tc.tile_pool(name=f"auxL{layer}", bufs=2, space="PSUM") as aux_ps,
    ):
        # adq: [128, qper*heads] per-dst attention values (local rows)
        adq_raw = wp.tile([QR, qper, heads], gdt, tag="adqr")
        nc.sync.dma_start(
            adq_raw[:], adst_src_r[:, 0:qper, adst_off:adst_off + heads])
        if gdt == BF16:
            adq_t = adq_raw
        else:
            adq_t = wp.tile([QR, qper, heads], BF16, tag="adq")
            nc.scalar.copy(adq_t[:], adq_raw[:])

        pend = {}   # gi -> O_tiles dict

        def bank_phase(gi, grp):
            O_tiles = {}
            for b in range(nbanks):
                blk = gi * nbanks + b
                sc = sched[blk]
                S, ncb, incs = sc["S"], nc_list[b], sc["incs"]
                SC = S * ncb
                nidx = SC * QR

                it = ixp.tile([QR, nidx // 16], I16, tag="idx")
                nc.sync.dma_start(
                    it[:], gidx_in[:, idx_off[blk] // 16:
                                   idx_off[blk + 1] // 16])
                G = gp.tile([QR, SC, row_w], gdt, tag="G")
                bs = meta["bank_start"][b]
                nc.gpsimd.dma_gather(
                    out_ap=G[:],
                    in_ap=table[bs:bs + bank_rows[b], :],
                    idxs_ap=it[:],
                    num_idxs=nidx, num_idxs_reg=nidx, elem_size=row_w,
                    single_packet=False)

                ninc = len(incs)
                p2t = pp2.tile([QR, ninc * QR], BF16, tag="p2")
                nc.sync.dma_start(
                    p2t[:], p2_in[:, p1_off[blk]:p1_off[blk + 1]])

                # adst per packed row: adp[:, s*heads+h] = sum_kk P2 @ adq
                adp = ad_ps.tile([QR, maxS * heads], F32, tag="adp")
                for i, (s, j, first, last) in enumerate(incs):
                    nc.tensor.matmul(
                        adp[:, s * heads:(s + 1) * heads],
                        p2t[:, i * QR:(i + 1) * QR],
                        adq_t[:, grp[0] + j, :],
                        start=first, stop=last)
                ads = wp.tile([QR, maxS * heads], F32, tag="ads")
                nc.scalar.copy(ads[:, 0:S * heads], adp[:, 0:S * heads])

                # e = prelu(asrc + ads) ; p = exp(e) ; msg *= p
                for s in range(S):
                    for h in range(heads):
                        nc.scalar.activation(
                            G[:, s * ncb:(s + 1) * ncb, feat + h],
                            G[:, s * ncb:(s + 1) * ncb, feat + h],
                            AF.Prelu,
                            bias=ads[:, s * heads + h:s * heads + h + 1],
                            alpha=NEG)
                for h in range(heads):
                    nc.scalar.activation(
                        G[:, :, feat + h], G[:, :, feat + h], AF.Exp)
                for h in range(heads):
                    nc.vector.tensor_tensor(
                        out=G[:, :, h * hw:(h + 1) * hw],
                        in0=G[:, :, h * hw:(h + 1) * hw],
                        in1=G[:, :, feat + h, None].broadcast_to(
                            [QR, SC, hw]),
                        op=ALU.mult)

                # O[:, s, :] = per-row sums of [msg | p]
                O = op.tile([QR, S, ocols], BF16, tag=f"O{b}")
                with nc.allow_low_precision(reason="bf16 partial sums"):
                    nc.vector.tensor_reduce(
                        out=O[:],
                        in_=G[:].rearrange("p (s j) f -> p s f j", j=ncb)
                             [:, :, 0:ocols, :],
                        axis=mybir.AxisListType.X, op=ALU.add)
                O_tiles[b] = O
            pend[gi] = O_tiles

        def combine_phase(gi, grp):
            ng = len(grp)
            O_tiles = pend.pop(gi)
            p1ts = {}
            for b in range(nbanks):
                blk = gi * nbanks + b
                ninc = len(sched[blk]["incs"])
                p1t = pp.tile([QR, ninc * QR], BF16, tag="p1")
                nc.sync.dma_start(
                    p1t[:], p1_in[:, p1_off[blk]:p1_off[blk + 1]])
                p1ts[b] = p1t
            # local rows for on-chip self-loop handling
            lcols = feat + 2 * heads
            Lq_raw = wp.tile([QR, ng, lcols], gdt, tag="Lqr")
            nc.sync.dma_start(
                Lq_raw[:], adst_src_r[:, grp[0]:grp[-1] + 1, 0:lcols])
            if gdt == F32:
                Lq = Lq_raw
            else:
                Lq = wp.tile([QR, ng, lcols], F32, tag="Lq")
                nc.scalar.copy(Lq[:], Lq_raw[:])
            if layer == 1:
                stg = wp.tile([QR, ng, FOUT + 2], BF16, tag="stB")
            else:
                stg = wp.tile([QR, ng, FOUT], F32, tag="stO")
            for j in range(ng):
                psq = cmb_ps.tile([QR, ocols], F32, tag="psq")
                started = False
                for b in range(nbanks):
                    incs = sched[gi * nbanks + b]["incs"]
                    p1t = p1ts[b]
                    O = O_tiles[b]
                    for i, (s, jj, _, _) in enumerate(incs):
                        if jj != j:
                            continue
                        nc.tensor.matmul(
                            psq[:], p1t[:, i * QR:(i + 1) * QR],
                            O[:, s, :], start=not started, stop=False)
                        started = True
                nc.tensor.matmul(psq[:], ones_t[:], bias_t[:],
                                 start=not started, stop=True)

                # self-loop: p_self = exp(prelu(asrc_own + adst_own))
                ps_self = wp.tile([QR, heads], F32, tag="pself")
                for h in range(heads):
                    nc.scalar.activation(
                        ps_self[:, h:h + 1], Lq[:, j, feat + h:feat + h + 1],
                        AF.Prelu,
                        bias=Lq[:, j, feat + heads + h:feat + heads + h + 1],
                        alpha=NEG)
                nc.scalar.activation(ps_self[:], ps_self[:], AF.Exp)
                sden = wp.tile([QR, heads], F32, tag="sden")
                nc.vector.tensor_tensor(
                    out=sden[:], in0=psq[:, feat:feat + heads],
                    in1=ps_self[:], op=ALU.add)
                msum = wp.tile([QR, feat], F32, tag="msum")
                nc.vector.tensor_tensor(
                    out=msum[:].rearrange("p (h f) -> p h f", h=heads),
                    in0=Lq[:, j, 0:feat].rearrange("p (h f) -> p h f",
                                                   h=heads),
                    in1=ps_self[:, :, None].broadcast_to([QR, heads, hw]),
                    op=ALU.mult)
                nc.vector.tensor_tensor(
                    out=msum[:], in0=msum[:], in1=psq[:, 0:feat],
                    op=ALU.add)

                rs = wp.tile([QR, heads], F32, tag="rs")
                nc.vector.tensor_scalar(
                    out=rs[:], in0=sden[:],
                    scalar1=1e-30, scalar2=None, op0=ALU.max)
                nc.vector.reciprocal(rs[:], rs[:])
                if layer == 1:
                    ot = wp.tile([QR, feat], BF16, tag="ot")
                    nc.vector.tensor_tensor(
                        out=ot[:].rearrange("p (h f) -> p h f", h=heads),
                        in0=msum[:].rearrange("p (h f) -> p h f",
                                              h=heads),
                        in1=rs[:, :, None].broadcast_to([QR, heads, hw]),
                        op=ALU.mult)
                    nc.scalar.activation(ot[:], ot[:], AF.Relu)
                    psT = aux_ps.tile([D1, QR], BF16, tag="psT")
                    nc.tensor.transpose(psT[:], ot[:], ident_t[:])
                    h1T = wp.tile([D1, QR], BF16, tag="h1T")
                    nc.scalar.copy(h1T[:], psT[:])
                    ps2 = aux_ps.tile([QR, FOUT + 2], F32, tag="ps2")
                    nc.tensor.matmul(ps2[:], h1T[:], Wext2_t[:],
                                     start=True, stop=True)
                    nc.scalar.copy(stg[:, j, :], ps2[:])
                else:
                    nc.vector.tensor_tensor(
                        out=stg[:, j, :],
                        in0=msum[:],
                        in1=rs[:, 0, None].broadcast_to([QR, feat]),
                        op=ALU.mult)
            if layer == 1:
                nc.scalar.dma_start(
                    bB_r[:, grp[0]:grp[-1] + 1, 0:FOUT + 2], stg[:])
                if ag_hook is not None:
                    ag_hook(gi)
            else:
                nc.scalar.dma_start(
                    out_r[:, grp[0]:grp[-1] + 1, :], stg[:])

        # software pipeline: group g's combine is emitted after group g+1's
        # gathers, so no gather ever waits (via cumulative DMA semaphores)
        # on the previous group's output write.
        for gi, grp in enumerate(groups):
            bank_phase(gi, grp)
            if gi > 0:
                combine_phase(gi - 1, groups[gi - 1])
        combine_phase(len(groups) - 1, groups[-1])


def kernel(x, edge_index, W1, att_src1, att_dst1, b1, W2, att_src2, att_dst2,
           b2):
    import time as _time
    _t = _time.time()
    in_maps, meta = preprocess(x, edge_index, W1, att_src1, att_dst1, b1,
                               W2, att_src2, att_dst2, b2)
    print(f"[kernel] preprocess {_time.time() - _t:.1f}s "
          f"(n_idx={meta['n_idx']}, nc={meta['nc_list']})", flush=True)
    _t = _time.time()
    nc = bacc.Bacc("TRN2", num_devices=NCORES, target_bir_lowering=False)
    build(nc, meta)
    print(f"[kernel] build {_time.time() - _t:.1f}s "
          f"({len(nc.inst_map)} inst)", flush=True)
    _t = _time.time()
    nc.compile()
    print(f"[kernel] bacc compile {_time.time() - _t:.1f}s", flush=True)
    _t = _time.time()
    trace = bool(os.environ.get("GAT_TRACE"))
    r = run_bass_kernel_spmd(nc, in_maps, list(range(NCORES)), trace=trace)
    print(f"[kernel] hw run {_time.time() - _t:.1f}s", flush=True)
    if trace and r.exec_time_ns is not None:
        print(f"HW exec time: {r.exec_time_ns} ns", flush=True)
    global _last_results, _last_meta, _last_inmaps
    _last_results, _last_meta, _last_inmaps = r, meta, in_maps
    shard = meta["shard"]
    full = np.concatenate([r.results[c]["out"] for c in range(NCORES)],
                          axis=0)
    out = full[meta["packed_of_node"]]
    return np.ascontiguousarray(out.astype(np.float32))
